# revision 13
# baseline (speedup 1.0000x reference)
"""Multi-head attention (B=4, S=2048, E=1024, H=16) on 8 TRN2 NeuronCores.

Sharding: core c -> (batch b = c//2, head-half hh = c%2  => 8 heads = 512 features).

v2 design (from trace analysis of the 698us baseline):
 - All DRAM inputs bf16 (halves DMA + LDWEIGHTS vs f32r).
 - V projection computed TRANSPOSED (x-tile stationary, W moving) so V lands
   directly in ctx-stationary layout [keys, head, dk] -- no PE transposes, no
   vector copies.  V bias folded into host-side bo' = bo + Wo @ bv (softmax
   normalization makes the V-bias term rowsum-invariant).
 - Exp split 23:9 between ScalarE (1 ACTIVATE/tile @ ~0.69us) and VectorE
   (2 custom DVE ops/tile @ ~1.36us) so both engines finish just under the
   PE's per-group time.
 - All PSUM evictions on ScalarE (activation Copy/Identity-with-bias);
   VectorE keeps only exp + reciprocal + normalize muls.
 - P2 kt-loop software-pipelined 1 deep (scores of kt+1 issued before ctx of
   kt) with scp=4/cxp=4 PSUM banks so the PE never waits on exp or bank
   recycling (PE pstate: only a continuously-busy PE runs at 0.42 ns/row).
 - Output projection as a separate final phase with its own 4 PSUM banks.
"""

import os
import sys

sys.path.insert(0, "/opt/trn_rl_repo")

import numpy as np

KDBG = bool(os.environ.get("KDBG"))

B, S, E, H = 4, 2048, 1024, 16
DK = E // H  # 64
NCORES = 8
F = 512  # features per core (head-half)
SCALE = 1.0 / 8.0  # 1/sqrt(DK)

# ---------------------------------------------------------------- helpers

_EXP_OPS = None


def _register_exp_ops():
    """Two custom DVE ops for exp(x/8) on raw scores |x| <= ~24:
    EXPA_ANT: q = (((c3*x + c2)*x + c1)*x + 1)^4  ~= exp(x/128)
    SQ4_ANT:  out = in^16  (4 squarings)  => exp(x/8).
    """
    global _EXP_OPS
    if _EXP_OPS is not None:
        return _EXP_OPS
    import concourse.dve_ops as dve_ops
    from concourse.dve_ops import DveOp, DveOpSpec, get_dve_sub_opcode
    from concourse.dve_spec import Spec, Src0, C0, C1, C2, One, sq, lower

    existing = {op.name: op for op in dve_ops.OPS}
    if "EXPA_ANT" in existing and "SQ4_ANT" in existing:
        _EXP_OPS = (existing["EXPA_ANT"], existing["SQ4_ANT"])
        return _EXP_OPS

    def _ref_a(in0, in1, c0, c1, c2):
        x = in0.astype(np.float32)
        q = ((x * np.float32(c2) + np.float32(c1)) * x + np.float32(c0)) * x + np.float32(1.0)
        q = q * q
        return q * q

    def _ref_sq4(in0, in1, c0, c1, c2):
        x = in0.astype(np.float32)
        for _ in range(4):
            x = x * x
        return x

    opa = DveOp(
        "EXPA_ANT",
        Spec(body=sq(sq(((Src0 * C2 + C1) * Src0 + C0) * Src0 + One)), reference=_ref_a),
        subdim=False,
        uops_sha={},
    )
    opb = DveOp(
        "SQ4_ANT",
        Spec(body=sq(sq(sq(sq(Src0)))), reference=_ref_sq4),
        subdim=False,
        uops_sha={},
    )
    for op in (opa, opb):
        dve_ops.OPS.append(op)
        dve_ops._SUB_OPCODE_FOR_NAME[op.name] = (
            max(dve_ops._SUB_OPCODE_FOR_NAME.values()) + 1
        )
        dve_ops.CUSTOM_DVE_SPECS[op.name] = op.spec
        for ver in ("v3", "v4"):
            try:
                spec_c = DveOpSpec(
                    name=op.name,
                    opcode=get_dve_sub_opcode(op.name),
                    uops=lower(op.spec, ver=ver),
                    rd1_en=False,
                )
                op.uops_sha[ver] = spec_c.sha(ver)
            except Exception:
                pass
    _EXP_OPS = (opa, opb)
    return _EXP_OPS


EXPA_CONSTS = {
    "s0": 1.0 / 512.0,
    "s1": 1.0 / (2.0 * 512.0**2),
    "imm2": 1.0 / (6.0 * 512.0**3),
}

# exp tiles per (qb,pr) group = 32; send these (evenly spread) to VectorE,
# the rest to ScalarE.  9/32 on vector, 23/32 on scalar.
_VSET = frozenset(
    i for i in range(32) if (i * 9) // 32 != ((i - 1) * 9) // 32
)

_BUILT = None  # cached compiled Bass program


def _build_program():
    global _BUILT
    if _BUILT is not None:
        return _BUILT

    import concourse.bass as bass
    import concourse.mybir as mybir
    from concourse import bacc
    from concourse.tile import TileContext

    EXPA, SQ4 = _register_exp_ops()

    F32 = mybir.dt.float32
    F32R = mybir.dt.float32r
    BF16 = mybir.dt.bfloat16
    AF = mybir.ActivationFunctionType

    nc = bacc.Bacc("TRN2", target_bir_lowering=False, debug=False, num_devices=NCORES)

    xq = nc.dram_tensor("xq", [E, S], BF16, kind="ExternalInput")
    xk = nc.dram_tensor("xk", [E, S], BF16, kind="ExternalInput")
    xv = nc.dram_tensor("xv", [E, S], BF16, kind="ExternalInput")
    wq = nc.dram_tensor("wq", [E, F], BF16, kind="ExternalInput")
    wk = nc.dram_tensor("wk", [E, F], BF16, kind="ExternalInput")
    wv = nc.dram_tensor("wv", [E, F], BF16, kind="ExternalInput")
    wo = nc.dram_tensor("wo", [F, E], BF16, kind="ExternalInput")
    bq = nc.dram_tensor("bq", [F], F32, kind="ExternalInput")
    bk = nc.dram_tensor("bk", [F], F32, kind="ExternalInput")
    out_d = nc.dram_tensor("out", [E, S], F32, kind="ExternalOutput")
    if KDBG:
        qt_d = nc.dram_tensor("qt_dbg", [128, 4, S], F32, kind="ExternalOutput")
        kt_d = nc.dram_tensor("kt_dbg", [128, 4, S], F32, kind="ExternalOutput")
        vn_d = nc.dram_tensor("vn_dbg", [128, 16, 8, 65], F32, kind="ExternalOutput")
        cx_d = nc.dram_tensor("cx_dbg", [128, 4, S], F32, kind="ExternalOutput")
        s0_d = nc.dram_tensor("s0_dbg", [128, 512], F32, kind="ExternalOutput")
        pt0_d = nc.dram_tensor("pt0_dbg", [128, 512], F32, kind="ExternalOutput")
        pt1_d = nc.dram_tensor("pt1_dbg", [128, 512], F32, kind="ExternalOutput")
        c0_d = nc.dram_tensor("c0_dbg", [65, 512], F32, kind="ExternalOutput")
        inv0_d = nc.dram_tensor("inv0_dbg", [1, 512], F32, kind="ExternalOutput")
        invB0_d = nc.dram_tensor("invB0_dbg", [64, 512], F32, kind="ExternalOutput")

    with TileContext(nc) as tc:
        with (
            tc.tile_pool(name="persist", bufs=1) as persist,
            tc.tile_pool(name="xp", bufs=3) as xp,
            tc.tile_pool(name="wp", bufs=1) as wp,
            tc.tile_pool(name="ptp", bufs=6) as ptp,
            tc.tile_pool(name="smp", bufs=3) as smp,
            tc.tile_pool(name="ost", bufs=3) as ostp,
        ):
            QT = persist.tile([128, 4, S], BF16)
            KT = persist.tile([128, 4, S], BF16)
            Vn = persist.tile([128, 16, 8, 65], BF16)
            CX = persist.tile([128, 4, S], BF16)

            biases = persist.tile([128, 2, 4], F32)
            for ti, bt in enumerate((bq, bk)):
                nc.sync.dma_start(
                    out=biases[:, ti, :], in_=bt.rearrange("(ft p) -> p ft", p=128)
                )
            # ones column for the rowsum trick (V stationary col 64)
            nc.vector.memset(Vn[:, :, :, 64:65], 1.0)

            # full weights in SBUF (bf16): [128, 8, F] for wq/wk/wv,
            # [128, 4, E] for wo
            wv_sb = wp.tile([128, 8, F], BF16, tag="wv")
            nc.sync.dma_start(out=wv_sb, in_=wv.rearrange("(ec p) f -> p ec f", p=128))
            wq_sb = wp.tile([128, 8, F], BF16, tag="wq")
            nc.sync.dma_start(out=wq_sb, in_=wq.rearrange("(ec p) f -> p ec f", p=128))
            wk_sb = wp.tile([128, 8, F], BF16, tag="wk")
            nc.sync.dma_start(out=wk_sb, in_=wk.rearrange("(ec p) f -> p ec f", p=128))
            wo_sb = wp.tile([128, 4, E], BF16, tag="wo")
            nc.sync.dma_start(out=wo_sb, in_=wo.rearrange("(fc p) e -> p fc e", p=128))

            # ---------------- P1: projections ----------------
            with tc.tile_pool(name="mm1", bufs=3, space="PSUM") as mm1:
                # V first: produced transposed ([s, f] = ctx-stationary layout)
                xv_r = xv.rearrange("(ec p) s -> p ec s", p=128)
                for sc in range(4):
                    ssl = slice(sc * 512, (sc + 1) * 512)
                    xch = xp.tile([128, 8, 512], BF16, tag="x")
                    nc.sync.dma_start(out=xch, in_=xv_r[:, :, ssl])
                    for st in range(4):
                        stsl = slice(st * 128, (st + 1) * 128)
                        p = mm1.tile([128, 512], F32, tag="mm")
                        for ec in range(8):
                            nc.tensor.matmul(
                                p,
                                xch[:, ec, stsl],
                                wv_sb[:, ec, :],
                                start=(ec == 0),
                                stop=(ec == 7),
                            )
                        kti = sc * 4 + st
                        nc.scalar.copy(
                            out=Vn[:, kti, :, 0:64],
                            in_=p.rearrange("p (h d) -> p h d", h=8),
                        )

                # Q, K: W stationary, x moving; bias added on eviction (ScalarE)
                for ti, (wsb, xt, dst) in enumerate(
                    ((wq_sb, xq, QT), (wk_sb, xk, KT))
                ):
                    xt_r = xt.rearrange("(ec p) s -> p ec s", p=128)
                    for sc in range(4):
                        ssl = slice(sc * 512, (sc + 1) * 512)
                        xch = xp.tile([128, 8, 512], BF16, tag="x")
                        nc.sync.dma_start(out=xch, in_=xt_r[:, :, ssl])
                        for ft in range(4):
                            fsl = slice(ft * 128, (ft + 1) * 128)
                            p = mm1.tile([128, 512], F32, tag="mm")
                            for ec in range(8):
                                nc.tensor.matmul(
                                    p,
                                    wsb[:, ec, fsl],
                                    xch[:, ec, :],
                                    start=(ec == 0),
                                    stop=(ec == 7),
                                )
                            nc.scalar.add(
                                out=dst[:, ft, ssl],
                                in_=p,
                                add=biases[:, ti, ft : ft + 1],
                            )

            # ---------------- P2: attention ----------------
            with (
                tc.tile_pool(name="scp", bufs=4, space="PSUM") as scp,
                tc.tile_pool(name="cxp", bufs=4, space="PSUM") as cxp,
            ):
                for qb in range(4):
                    qsl = slice(qb * 512, (qb + 1) * 512)
                    for pr in range(4):
                        c0 = cxp.tile([65, 512], F32, tag="cx")
                        c1 = cxp.tile([65, 512], F32, tag="cx")

                        def scores(kt):
                            ksl = slice(kt * 128, (kt + 1) * 128)
                            s0 = scp.tile([128, 512], F32, tag="sc", name=f"s0_{kt}")
                            s1 = scp.tile([128, 512], F32, tag="sc", name=f"s1_{kt}")
                            nc.tensor.matmul(
                                s0, KT[0:64, pr, ksl], QT[0:64, pr, qsl],
                                start=True, stop=True, tile_position=(0, 0),
                            )
                            nc.tensor.matmul(
                                s1, KT[64:128, pr, ksl], QT[64:128, pr, qsl],
                                start=True, stop=True, tile_position=(64, 0),
                            )
                            return s0, s1

                        def exp_tile(s, idx):
                            pt = ptp.tile([128, 512], BF16, tag="pt", name=f"pt_{idx}")
                            if idx in _VSET:
                                escr = ptp.tile(
                                    [128, 512], F32, tag="escr", bufs=2,
                                    name=f"escr_{idx}",
                                )
                                nc.vector._custom_dve(
                                    EXPA, out=escr, in0=s, **EXPA_CONSTS
                                )
                                nc.vector._custom_dve(SQ4, out=pt, in0=escr)
                            else:
                                nc.scalar.activation(
                                    out=pt, in_=s, func=AF.Exp, scale=SCALE
                                )
                            return pt

                        def dbg_dump(src, dst, nm):
                            dt = ostp.tile(
                                [src.shape[0], 512], F32, tag="dbg2", bufs=2,
                                name=f"dbg_{nm}",
                            )
                            nc.vector.tensor_copy(out=dt[0 : src.shape[0], :], in_=src)
                            nc.sync.dma_start(out=dst, in_=dt[0 : src.shape[0], :])

                        sq_ = scores(0)
                        for kt in range(16):
                            if KDBG and qb == 0 and pr == 0 and kt == 0:
                                dbg_dump(sq_[0], s0_d[:, :], "s0")
                            pt0 = exp_tile(sq_[0], 2 * kt)
                            pt1 = exp_tile(sq_[1], 2 * kt + 1)
                            if KDBG and qb == 0 and pr == 0 and kt == 0:
                                dbg_dump(pt0, pt0_d[:, :], "pt0")
                                dbg_dump(pt1, pt1_d[:, :], "pt1")
                            if kt < 15:
                                sq_ = scores(kt + 1)
                            nc.tensor.matmul(
                                c0, Vn[:, kt, 2 * pr, :], pt0,
                                start=(kt == 0), stop=(kt == 15),
                            )
                            nc.tensor.matmul(
                                c1, Vn[:, kt, 2 * pr + 1, :], pt1,
                                start=(kt == 0), stop=(kt == 15),
                            )

                        # normalize: CX[:, pr, qsl] = ctx / rowsum
                        if KDBG and qb == 0 and pr == 0:
                            dbg_dump(c0[0:65, :], c0_d[:, :], "c0")
                        sums = smp.tile([1, 1024], F32, tag="sums")
                        nc.vector.tensor_copy(out=sums[0:1, 0:512], in_=c0[64:65, :])
                        nc.vector.tensor_copy(out=sums[0:1, 512:1024], in_=c1[64:65, :])
                        inv = smp.tile([1, 1024], F32, tag="inv")
                        nc.vector.reciprocal_approx_fast(out=inv, in_=sums)
                        invB0 = smp.tile([64, 512], F32, tag="invB")
                        invB1 = smp.tile([64, 512], F32, tag="invB")
                        nc.gpsimd.partition_broadcast(out_ap=invB0, in_ap=inv[0:1, 0:512])
                        nc.gpsimd.partition_broadcast(out_ap=invB1, in_ap=inv[0:1, 512:1024])
                        if KDBG and qb == 0 and pr == 0:
                            dbg_dump(inv[0:1, :], inv0_d[:, :], "inv0")
                            dbg_dump(invB0, invB0_d[:, :], "invB0")
                        nc.vector.tensor_mul(CX[0:64, pr, qsl], c0[0:64, :], invB0)
                        nc.vector.tensor_mul(CX[64:128, pr, qsl], c1[0:64, :], invB1)

            if KDBG:
                for src, dst in (
                    (QT, qt_d[:, :, :]),
                    (KT, kt_d[:, :, :]),
                    (Vn, vn_d[:, :, :, :]),
                    (CX, cx_d[:, :, :]),
                ):
                    dbgt = ostp.tile(list(src.shape), F32, tag="dbg", bufs=1,
                                     name="dbgt")
                    nc.vector.tensor_copy(out=dbgt, in_=src)
                    nc.sync.dma_start(out=dst, in_=dbgt)

            # ---------------- P3: output projection ----------------
            with tc.tile_pool(name="mmo", bufs=4, space="PSUM") as mmo:
                for qb in range(4):
                    qsl = slice(qb * 512, (qb + 1) * 512)
                    for et in range(8):
                        esl = slice(et * 128, (et + 1) * 128)
                        p = mmo.tile([128, 512], F32, tag="mm")
                        for fc in range(4):
                            nc.tensor.matmul(
                                p, wo_sb[:, fc, esl], CX[:, fc, qsl],
                                start=(fc == 0), stop=(fc == 3),
                            )
                        o = ostp.tile([128, 512], F32, tag="ost")
                        nc.scalar.copy(out=o, in_=p)
                        nc.sync.dma_start(out=out_d[esl, qsl], in_=o)

    nc.compile()
    _BUILT = nc
    return nc


def _to_bf16(x: np.ndarray):
    import ml_dtypes

    return np.ascontiguousarray(x).astype(ml_dtypes.bfloat16)


def _make_in_maps(inputs):
    query = np.asarray(inputs["query"], dtype=np.float32)
    key_ = np.asarray(inputs["key_"], dtype=np.float32)
    value = np.asarray(inputs["value"], dtype=np.float32)
    Wq = np.asarray(inputs["Wq"], dtype=np.float32)
    bq = np.asarray(inputs["bq"], dtype=np.float32)
    Wk = np.asarray(inputs["Wk"], dtype=np.float32)
    bk = np.asarray(inputs["bk"], dtype=np.float32)
    Wv = np.asarray(inputs["Wv"], dtype=np.float32)
    Wo = np.asarray(inputs["Wo"], dtype=np.float32)

    WqT = _to_bf16(Wq.T)  # [E_in, E_out]
    WkT = _to_bf16(Wk.T)
    WvT = _to_bf16(Wv.T)
    WoT = _to_bf16(Wo.T)  # [F_in, E_out]

    in_maps = []
    for c in range(NCORES):
        b = c // 2
        hh = c % 2
        fsl = slice(hh * F, (hh + 1) * F)
        in_maps.append(
            {
                "xq": _to_bf16(query[b].T),
                "xk": _to_bf16(key_[b].T),
                "xv": _to_bf16(value[b].T),
                "wq": np.ascontiguousarray(WqT[:, fsl]),
                "wk": np.ascontiguousarray(WkT[:, fsl]),
                "wv": np.ascontiguousarray(WvT[:, fsl]),
                "wo": np.ascontiguousarray(WoT[fsl, :]),
                "bq": np.ascontiguousarray(bq[fsl]),
                "bk": np.ascontiguousarray(bk[fsl]),
            }
        )
    return in_maps


def kernel(**inputs) -> np.ndarray:
    from concourse.bass_utils import run_bass_kernel_spmd

    nc = _build_program()
    in_maps = _make_in_maps(inputs)

    bv = np.asarray(inputs["bv"], dtype=np.float32)
    bo = np.asarray(inputs["bo"], dtype=np.float32)
    Wo = np.asarray(inputs["Wo"], dtype=np.float32)
    bo_prime = bo + Wo @ bv  # V-bias folded through softmax + out-proj

    res = run_bass_kernel_spmd(nc, in_maps, core_ids=list(range(NCORES)))

    out = np.empty((B, S, E), dtype=np.float32)
    for b in range(B):
        partial = res.results[2 * b]["out"] + res.results[2 * b + 1]["out"]  # [E, S]
        out[b] = partial.T + bo_prime[None, :]
    return out
